# revision 9
# baseline (speedup 1.0000x reference)
"""Trainium2 Bass kernel: dense attention with key-padding mask (ColoAttention).

Math (per batch b, head h):
    scores = (Q @ K^T) / sqrt(D); masked keys -> -inf; softmax over keys;
    out = probs @ V; rows at masked query positions zeroed.

Implementation notes:
  - The mask is a contiguous valid prefix per batch (ragged sequences).  The
    host derives per-batch valid extents and compiles a program that only
    touches the valid key chunks / query columns (~50% of the dense work for
    the reference distribution).  Programs are cached per extent tuple.
  - Sharding balances the ragged work: every core gets 2 heads from EVERY
    batch (16 heads / 8 cores), so all cores run the identical schedule.
  - K and V rows at masked key positions are zeroed on the host, so scores
    at masked keys inside the last partial chunk are exactly 0, exp(0) = 1,
    and the per-row sum of exponentials just needs the (host-known)
    masked-key count subtracted.  Masked keys contribute 0 to probs @ V.
  - Scores are computed transposed (S^T[k, q] = K @ Q^T) so the exp output
    P^T[k, q] (bf16) is directly the moving operand for O'^T = V^T @ P^T.
  - Row sums: P^T chunks are pairwise tree-accumulated over k-chunks on the
    DVE (bf16); a ones-vector matmul reduces the tree result (all chunks but
    the last) plus the last chunk directly, accumulated in PSUM, so the PE
    never waits on the post-last-chunk DVE fold.
  - The device emits unnormalized O'^T and raw sums; the host applies
    qmask/(sums - mcount) and the final [D,S]->[S,D] transpose.
  - QK^T runs in float32r (full-rate fp32 on the PE), PV in bf16.
  - PSUM budget (8 banks): scores [128,1024] x2 = 4, O' accum [128,1024]
    x1 = 2, sums [1,512] x2 = 2.
"""

import numpy as np
import ml_dtypes
from contextlib import ExitStack

import concourse.bass as bass
import concourse.mybir as mybir
import concourse.tile as tile
from concourse import bacc
from concourse.bass_utils import run_bass_kernel_spmd

B, S, H, D = 4, 2048, 16, 128
N_CORES = 8
P = 128
SCALE = 1.0 / float(np.sqrt(np.float64(D)).astype(np.float32))


def _subs_of(qw: int):
    """Split the valid query width into <=1024-wide sub-blocks, multiples of
    256 so QK windows are 512-sized or a >=256 remainder (full-rate f32r)."""
    if qw <= 1024:
        return [(0, qw)]
    wa = min(1024, ((qw // 2 + 255) // 256) * 256)
    return [(0, wa), (wa, qw - wa)]


def _mm_windows(w: int):
    """Matmul windows covering [0, w): 512-wide (one PSUM bank) plus a
    remainder window."""
    wins = []
    for ws in range(0, w, 512):
        wins.append((ws, 1, min(512, w - ws)))
    return wins


def build_program(sched) -> bacc.Bacc:
    """sched: tuple of (nkc, qw) per slot, identical on every core."""
    f32 = mybir.dt.float32
    f32r = mybir.dt.float32r
    bf16 = mybir.dt.bfloat16
    Exp = mybir.ActivationFunctionType.Exp

    nc = bacc.Bacc("TRN2", target_bir_lowering=False, debug=False)
    q_d = nc.dram_tensor("q", [8, P, S], f32r, kind="ExternalInput").ap()
    k_d = nc.dram_tensor("k", [8, P, S], f32r, kind="ExternalInput").ap()
    v_d = nc.dram_tensor("v", [8, S, P], bf16, kind="ExternalInput").ap()
    out_d = nc.dram_tensor("out", [8, P, S], f32, kind="ExternalOutput").ap()
    sums_d = nc.dram_tensor("sums_out", [8, S], f32, kind="ExternalOutput").ap()

    # steps: (slot, sub_off, sub_w, kc)
    steps = []
    for s, (nkc, qw) in enumerate(sched):
        for (off, w) in _subs_of(qw):
            for kc in range(nkc):
                steps.append((s, off, w, kc))

    def mm(out_ap, lhsT, rhs, n, w, start, stop):
        nc.tensor.matmul(out_ap, lhsT=lhsT, rhs=rhs, start=start, stop=stop)

    with tile.TileContext(nc) as tc:
        with ExitStack() as ctx:
            consts = ctx.enter_context(tc.tile_pool(name="consts", bufs=1))
            qkp = ctx.enter_context(tc.tile_pool(name="qkp", bufs=2))
            ptp = ctx.enter_context(tc.tile_pool(name="ptp", bufs=4))
            treep = ctx.enter_context(tc.tile_pool(name="treep", bufs=8))
            otp = ctx.enter_context(tc.tile_pool(name="otp", bufs=3))
            smp = ctx.enter_context(tc.tile_pool(name="smp", bufs=4))
            sps = ctx.enter_context(tc.tile_pool(name="sps", bufs=2, space="PSUM"))
            ops = ctx.enter_context(tc.tile_pool(name="ops", bufs=1, space="PSUM"))
            smps = ctx.enter_context(tc.tile_pool(name="smps", bufs=2, space="PSUM"))

            ones_b = consts.tile([P, 1], bf16, tag="ones")
            nc.gpsimd.memset(ones_b[:], 1.0)

            staged = {}

            def stage(s):
                if s in staged or s >= len(sched):
                    return
                nkc, qw = sched[s]
                q_sb = qkp.tile([P, qw], f32r, tag="q", name=f"q_{s}")
                k_sb = qkp.tile([P, nkc * P], f32r, tag="k", name=f"k_{s}")
                v_sb = qkp.tile([P, nkc, P], bf16, tag="v", name=f"v_{s}")
                # first compute needs k/v chunk 0 and the first q window
                nc.sync.dma_start(k_sb[:, 0:P], k_d[s, :, 0:P])
                nc.sync.dma_start(q_sb[:, 0:512], q_d[s, :, 0:512])
                nc.sync.dma_start(v_sb[:, 0, :], v_d[s, 0:P, :])
                for ws in range(512, qw, 512):
                    we = min(qw, ws + 512)
                    nc.sync.dma_start(q_sb[:, ws:we], q_d[s, :, ws:we])
                for kc in range(1, nkc):
                    nc.sync.dma_start(k_sb[:, kc * P:(kc + 1) * P],
                                      k_d[s, :, kc * P:(kc + 1) * P])
                    nc.sync.dma_start(v_sb[:, kc, :],
                                      v_d[s, kc * P:(kc + 1) * P, :])
                staged[s] = (q_sb, k_sb, v_sb)

            stage(0)
            stage(1)

            def emit_qk(i):
                s, off, w, kc = steps[i]
                stage(s + 1)
                q_sb, k_sb, _ = staged[s]
                s_ps = sps.tile([P, 1024], f32, tag="s", name=f"s_{i}")
                for (ws, n, ww) in _mm_windows(w):
                    mm(s_ps[:, ws:ws + n * ww],
                       k_sb[:, kc * P:(kc + 1) * P],
                       q_sb[:, off + ws:off + ws + n * ww],
                       n, ww, True, True)
                return s_ps

            # binary-counter tree accumulation of exp chunks (per sub-block)
            levels = [None] * 6

            def tree_push(pt, w, i):
                cur, lvl = pt, 0
                while levels[lvl] is not None:
                    prev = levels[lvl]
                    levels[lvl] = None
                    nt = treep.tile([P, 1024], bf16, tag="tree",
                                    name=f"tr_{i}_{lvl}")
                    nc.vector.tensor_add(nt[:, :w], prev[:, :w], cur[:, :w])
                    cur = nt
                    lvl += 1
                levels[lvl] = cur

            def tree_fold_partial(w, i):
                acc = None
                for lvl in range(6):
                    if levels[lvl] is None:
                        continue
                    if acc is None:
                        acc = levels[lvl]
                    else:
                        nt = treep.tile([P, 1024], bf16, tag="tree",
                                        name=f"tf_{i}_{lvl}")
                        nc.vector.tensor_add(nt[:, :w], acc[:, :w],
                                             levels[lvl][:, :w])
                        acc = nt
                    levels[lvl] = None
                return acc

            pend = {0: emit_qk(0)}
            ot_ps = None
            prefold = None
            for i, (s, off, w, kc) in enumerate(steps):
                nkc, qw = sched[s]
                q_sb, k_sb, v_sb = staged[s]
                if kc == 0:
                    ot_ps = ops.tile([P, 1024], f32, tag="o", name=f"ot_{i}")
                    prefold = None
                pt = ptp.tile([P, 1024], bf16, tag="pt", name=f"pt_{i}")
                s_ps = pend.pop(i)
                nc.scalar.activation(pt[:, :w], s_ps[:, :w], Exp, scale=SCALE)
                if i + 1 < len(steps):
                    pend[i + 1] = emit_qk(i + 1)
                last = kc == nkc - 1
                if last:
                    # sums: ones @ prefold (ready early) + ones @ pt_last,
                    # PSUM-accumulated per bank-sized window
                    sm_tiles = []
                    for (ws, n, ww) in _mm_windows(w):
                        for j in range(n):
                            wo = ws + j * ww
                            wl = min(ww, w - wo)
                            sm_ps = smp_tile = smps.tile(
                                [1, 512], f32, tag="sm", name=f"sm_{i}_{wo}")
                            if prefold is not None:
                                nc.tensor.matmul(
                                    sm_ps[:, :wl], lhsT=ones_b[:],
                                    rhs=prefold[:, wo:wo + wl],
                                    start=True, stop=False)
                            sm_tiles.append((sm_ps, wo, wl))
                for (ws, n, ww) in _mm_windows(w):
                    mm(ot_ps[:, ws:ws + n * ww],
                       v_sb[:, kc, :],
                       pt[:, ws:ws + n * ww],
                       n, ww, kc == 0, last)
                if not last:
                    tree_push(pt, w, i)
                    if kc == nkc - 2:
                        prefold = tree_fold_partial(w, i)
                    continue

                # ---- sub-block tail ----
                for (sm_ps, wo, wl) in sm_tiles:
                    nc.tensor.matmul(sm_ps[:, :wl], lhsT=ones_b[:],
                                     rhs=pt[:, wo:wo + wl],
                                     start=(prefold is None), stop=True)
                o_st = otp.tile([P, 1024], f32, tag="ost", name=f"ost_{i}")
                nc.vector.tensor_copy(out=o_st[:, :w], in_=ot_ps[:, :w])
                nc.sync.dma_start(out_d[s, :, off:off + w], o_st[:, :w])
                for (sm_ps, wo, wl) in sm_tiles:
                    sm_st = smp.tile([1, 512], f32, tag="smst",
                                     name=f"smst_{i}_{wo}")
                    nc.vector.tensor_copy(out=sm_st[:, :wl],
                                          in_=sm_ps[:, :wl])
                    nc.sync.dma_start(sums_d[s, off + wo:off + wo + wl],
                                      sm_st[:, :wl])

    nc.compile()
    return nc


_PROG_CACHE: dict = {}


def _get_program(sched) -> bacc.Bacc:
    if sched not in _PROG_CACHE:
        _PROG_CACHE[sched] = build_program(sched)
    return _PROG_CACHE[sched]


def _plan(attn_mask):
    mf = (np.asarray(attn_mask) > 0)
    any_valid = mf.any(axis=1)
    last_plus1 = np.where(any_valid, S - np.argmax(mf[:, ::-1], axis=1), 1)
    nkc = np.maximum(1, (last_plus1 + P - 1) // P).astype(int)   # [B]
    qw = nkc * P
    # slot s -> batch s//2 (two head-slots per batch per core)
    sched = tuple((int(nkc[s // 2]), int(qw[s // 2])) for s in range(8))
    return mf, nkc, qw, sched


def make_in_maps(query, key, value, attn_mask):
    mf, nkc, qw, sched = _plan(attn_mask)
    # device wants q/k as [slot, D, S] (pre-transposed), v as [slot, S, D]
    qT = np.asarray(query, np.float32).transpose(0, 2, 3, 1)     # [B, H, D, S]
    kT = np.asarray(key, np.float32).transpose(0, 2, 3, 1)       # [B, H, D, S]
    v = np.asarray(value, np.float32).transpose(0, 2, 1, 3)      # [B, H, S, D]
    mff = mf.astype(np.float32)
    kTz = kT * mff[:, None, None, :]
    vz = (v * mff[:, None, :, None]).astype(ml_dtypes.bfloat16)
    in_maps = []
    for c in range(N_CORES):
        qs = np.empty((8, P, S), np.float32)
        ks = np.empty((8, P, S), np.float32)
        vs = np.empty((8, S, P), ml_dtypes.bfloat16)
        for s in range(8):
            b, h = s // 2, 2 * c + (s % 2)
            w = qw[b]
            qs[s, :, :w] = qT[b, h, :, :w]
            ks[s, :, :w] = kTz[b, h, :, :w]
            vs[s, :w, :] = vz[b, h, :w, :]
        in_maps.append({"q": qs, "k": ks, "v": vs})
    return in_maps, mf


def assemble_output(results, mf):
    _, nkc, qw, _ = _plan(mf.astype(np.int32))
    mcount = np.array([nkc[b] * P - mf[b, :nkc[b] * P].sum() for b in range(B)],
                      np.float32)
    out = np.zeros((B, S, H * D), np.float32)
    for c in range(N_CORES):
        for s in range(8):
            b, h = s // 2, 2 * c + (s % 2)
            w = int(qw[b])
            oT = results[c]["out"][s][:, :w]                     # [D, w]
            sums = results[c]["sums_out"][s][:w] - mcount[b]     # [w]
            with np.errstate(divide="ignore", invalid="ignore"):
                scale = np.where(mf[b, :w], 1.0 / sums, 0.0)
            out[b, :w, h * D:(h + 1) * D] = (oT * scale[None, :]).T
    return out


def kernel(query, key, value, attn_mask):
    _, _, _, sched = _plan(attn_mask)
    nc = _get_program(sched)
    in_maps, mf = make_in_maps(query, key, value, attn_mask)
    res = run_bass_kernel_spmd(nc, in_maps, list(range(N_CORES)))
    return assemble_output(res.results, mf)


# revision 10
# speedup vs baseline: 1.1035x; 1.1035x over previous
"""Trainium2 Bass kernel: dense attention with key-padding mask (ColoAttention).

Math (per batch b, head h):
    scores = (Q @ K^T) / sqrt(D); masked keys -> -inf; softmax over keys;
    out = probs @ V; rows at masked query positions zeroed.

Implementation notes:
  - The mask is a contiguous valid prefix per batch (ragged sequences).  The
    host derives per-batch valid extents and compiles a program that only
    touches the valid key chunks / query columns (~50% of the dense work for
    the reference distribution).  Programs are cached per extent tuple.
  - Sharding balances the ragged work: every core gets 2 heads from EVERY
    batch (16 heads / 8 cores), so all cores run the identical schedule.
  - K and V rows at masked key positions are zeroed on the host, so scores
    at masked keys inside the last partial chunk are exactly 0, exp(0) = 1,
    and the per-row sum of exponentials just needs the (host-known)
    masked-key count subtracted.  Masked keys contribute 0 to probs @ V.
  - Scores are computed transposed (S^T[k, q] = K @ Q^T) so the exp output
    P^T[k, q] (bf16) is directly the moving operand for O'^T = V^T @ P^T.
  - Row sums: P^T chunks are pairwise tree-accumulated over k-chunks on the
    DVE (bf16), then one ones-vector matmul per q-sub-block reduces the 128
    partitions exactly in PSUM.  The PE stream is 2 passes of the score
    matrix (QK + PV) instead of 3.  The sums matmul + drain are deferred by
    one pipeline step so the PE never waits on the DVE fold; a dummy s-pool
    slot keeps the score-buffer rotation parity intact.
  - The device emits unnormalized O'^T and raw sums; the host applies
    qmask/(sums - mcount) and the final [D,S]->[S,D] transpose.
  - QK^T runs in float32r (full-rate fp32 on the PE), PV in bf16.
  - PSUM (8 banks): scores [128,1024] x2 = 4 banks (sums matmul rides this
    rotation), O' accum [128,1024] x2 = 4 banks.
"""

import numpy as np
import ml_dtypes
from contextlib import ExitStack

import concourse.bass as bass
import concourse.mybir as mybir
import concourse.tile as tile
from concourse import bacc
from concourse.bass_utils import run_bass_kernel_spmd

B, S, H, D = 4, 2048, 16, 128
N_CORES = 8
P = 128
SCALE = 1.0 / float(np.sqrt(np.float64(D)).astype(np.float32))


def _subs_of(qw: int):
    """Split the valid query width into <=1024-wide sub-blocks, multiples of
    256 so QK windows are 512-sized or a >=256 remainder (full-rate f32r)."""
    if qw <= 1024:
        return [(0, qw)]
    wa = min(1024, ((qw // 2 + 255) // 256) * 256)
    return [(0, wa), (wa, qw - wa)]


def _mm_windows(w: int):
    """512-wide (PSUM-bank sized) matmul windows covering [0, w)."""
    return [(ws, min(512, w - ws)) for ws in range(0, w, 512)]


def build_program(sched) -> bacc.Bacc:
    """sched: tuple of (nkc, qw) per slot, identical on every core."""
    f32 = mybir.dt.float32
    f32r = mybir.dt.float32r
    bf16 = mybir.dt.bfloat16
    Exp = mybir.ActivationFunctionType.Exp

    nc = bacc.Bacc("TRN2", target_bir_lowering=False, debug=False)
    q_d = nc.dram_tensor("q", [8, P, S], f32r, kind="ExternalInput").ap()
    k_d = nc.dram_tensor("k", [8, P, S], f32r, kind="ExternalInput").ap()
    v_d = nc.dram_tensor("v", [8, S, P], bf16, kind="ExternalInput").ap()
    out_d = nc.dram_tensor("out", [8, P, S], f32, kind="ExternalOutput").ap()
    sums_d = nc.dram_tensor("sums_out", [8, S], f32, kind="ExternalOutput").ap()

    # steps: (slot, sub_off, sub_w, kc)
    steps = []
    for s, (nkc, qw) in enumerate(sched):
        for (off, w) in _subs_of(qw):
            for kc in range(nkc):
                steps.append((s, off, w, kc))

    with tile.TileContext(nc) as tc:
        with ExitStack() as ctx:
            consts = ctx.enter_context(tc.tile_pool(name="consts", bufs=1))
            qkp = ctx.enter_context(tc.tile_pool(name="qkp", bufs=2))
            ptp = ctx.enter_context(tc.tile_pool(name="ptp", bufs=4))
            treep = ctx.enter_context(tc.tile_pool(name="treep", bufs=8))
            otp = ctx.enter_context(tc.tile_pool(name="otp", bufs=4))
            smp = ctx.enter_context(tc.tile_pool(name="smp", bufs=2))
            sps = ctx.enter_context(tc.tile_pool(name="sps", bufs=2, space="PSUM"))
            ops = ctx.enter_context(tc.tile_pool(name="ops", bufs=2, space="PSUM"))

            ones_b = consts.tile([P, 1], bf16, tag="ones")
            nc.gpsimd.memset(ones_b[:], 1.0)

            staged = {}

            def stage(s):
                if s in staged or s >= len(sched):
                    return
                nkc, qw = sched[s]
                q_sb = qkp.tile([P, qw], f32r, tag="q", name=f"q_{s}")
                k_sb = qkp.tile([P, nkc * P], f32r, tag="k", name=f"k_{s}")
                v_sb = qkp.tile([P, nkc, P], bf16, tag="v", name=f"v_{s}")
                # first compute needs k/v chunk 0 and the first q window
                nc.sync.dma_start(k_sb[:, 0:P], k_d[s, :, 0:P])
                nc.sync.dma_start(q_sb[:, 0:512], q_d[s, :, 0:512])
                nc.sync.dma_start(v_sb[:, 0, :], v_d[s, 0:P, :])
                for ws in range(512, qw, 512):
                    we = min(qw, ws + 512)
                    nc.sync.dma_start(q_sb[:, ws:we], q_d[s, :, ws:we])
                for kc in range(1, nkc):
                    nc.sync.dma_start(k_sb[:, kc * P:(kc + 1) * P],
                                      k_d[s, :, kc * P:(kc + 1) * P])
                    nc.sync.dma_start(v_sb[:, kc, :],
                                      v_d[s, kc * P:(kc + 1) * P, :])
                staged[s] = (q_sb, k_sb, v_sb)

            stage(0)
            stage(1)

            def emit_qk(i):
                s, off, w, kc = steps[i]
                stage(s + 1)
                q_sb, k_sb, _ = staged[s]
                s_ps = sps.tile([P, 1024], f32, tag="s", name=f"s_{i}")
                for (ws, ww) in _mm_windows(w):
                    nc.tensor.matmul(
                        s_ps[:, ws:ws + ww],
                        lhsT=k_sb[:, kc * P:(kc + 1) * P],
                        rhs=q_sb[:, off + ws:off + ws + ww],
                        start=True, stop=True)
                return s_ps

            # binary-counter tree accumulation of exp chunks (per sub-block)
            levels = [None] * 6

            def tree_push(pt, w, i):
                cur, lvl = pt, 0
                while levels[lvl] is not None:
                    prev = levels[lvl]
                    levels[lvl] = None
                    nt = treep.tile([P, 1024], bf16, tag="tree",
                                    name=f"tr_{i}_{lvl}")
                    nc.vector.tensor_add(nt[:, :w], prev[:, :w], cur[:, :w])
                    cur = nt
                    lvl += 1
                levels[lvl] = cur

            def tree_fold_partial(w, i):
                acc = None
                for lvl in range(6):
                    if levels[lvl] is None:
                        continue
                    if acc is None:
                        acc = levels[lvl]
                    else:
                        nt = treep.tile([P, 1024], bf16, tag="tree",
                                        name=f"tf_{i}_{lvl}")
                        nc.vector.tensor_add(nt[:, :w], acc[:, :w],
                                             levels[lvl][:, :w])
                        acc = nt
                    levels[lvl] = None
                return acc

            def make_tail(s, off, w, acc, i):
                """Deferred sums matmul + drains for a finished sub-block.
                Emitted one pipeline step later so the PE never waits on the
                DVE fold chain."""
                def tail():
                    sm_ps = sps.tile([P, 1024], f32, tag="s", name=f"sm_{i}")
                    # dummy allocation keeps the s rotation parity so the
                    # next QK lands on the buffer freed two exps ago
                    sps.tile([P, 1024], f32, tag="s", name=f"sdummy_{i}")
                    for (ws, ww) in _mm_windows(w):
                        nc.tensor.matmul(sm_ps[0:1, ws:ws + ww],
                                         lhsT=ones_b[:],
                                         rhs=acc[:, ws:ws + ww],
                                         start=True, stop=True)
                    sm_st = smp.tile([1, 1024], f32, tag="smst",
                                     name=f"smst_{i}")
                    nc.vector.tensor_copy(out=sm_st[:, :w],
                                          in_=sm_ps[0:1, :w])
                    nc.sync.dma_start(sums_d[s, off:off + w], sm_st[:, :w])
                return tail

            pend = {0: emit_qk(0)}
            pend_tail = None
            ot_ps = None
            prefold = None
            for i, (s, off, w, kc) in enumerate(steps):
                nkc, qw = sched[s]
                q_sb, k_sb, v_sb = staged[s]
                if kc == 0:
                    ot_ps = ops.tile([P, 1024], f32, tag="o", name=f"ot_{i}")
                    prefold = None
                pt = ptp.tile([P, 1024], bf16, tag="pt", name=f"pt_{i}")
                s_ps = pend.pop(i)
                nc.scalar.activation(pt[:, :w], s_ps[:, :w], Exp, scale=SCALE)
                if i + 1 < len(steps):
                    pend[i + 1] = emit_qk(i + 1)
                last = kc == nkc - 1
                for (ws, ww) in _mm_windows(w):
                    nc.tensor.matmul(
                        ot_ps[:, ws:ws + ww],
                        lhsT=v_sb[:, kc, :],
                        rhs=pt[:, ws:ws + ww],
                        start=(kc == 0), stop=last)
                if pend_tail is not None:
                    pend_tail()
                    pend_tail = None
                if not last:
                    tree_push(pt, w, i)
                    if kc == nkc - 2:
                        prefold = tree_fold_partial(w, i)
                    continue

                # ---- sub-block tail ----
                if nkc > 1:
                    acc = treep.tile([P, 1024], bf16, tag="tree",
                                     name=f"acc_{i}")
                    nc.vector.tensor_add(acc[:, :w], prefold[:, :w],
                                         pt[:, :w])
                else:
                    acc = pt
                o_st = otp.tile([P, 1024], f32, tag="ost", name=f"ost_{i}")
                nc.vector.tensor_copy(out=o_st[:, :w], in_=ot_ps[:, :w])
                nc.sync.dma_start(out_d[s, :, off:off + w], o_st[:, :w])
                pend_tail = make_tail(s, off, w, acc, i)
            if pend_tail is not None:
                pend_tail()

    nc.compile()
    return nc


_PROG_CACHE: dict = {}


def _get_program(sched) -> bacc.Bacc:
    if sched not in _PROG_CACHE:
        _PROG_CACHE[sched] = build_program(sched)
    return _PROG_CACHE[sched]


def _plan(attn_mask):
    mf = (np.asarray(attn_mask) > 0)
    any_valid = mf.any(axis=1)
    last_plus1 = np.where(any_valid, S - np.argmax(mf[:, ::-1], axis=1), 1)
    nkc = np.maximum(1, (last_plus1 + P - 1) // P).astype(int)   # [B]
    qw = nkc * P
    # slot s -> batch s//2 (two head-slots per batch per core)
    sched = tuple((int(nkc[s // 2]), int(qw[s // 2])) for s in range(8))
    return mf, nkc, qw, sched


def make_in_maps(query, key, value, attn_mask):
    mf, nkc, qw, sched = _plan(attn_mask)
    # device wants q/k as [slot, D, S] (pre-transposed), v as [slot, S, D]
    qT = np.asarray(query, np.float32).transpose(0, 2, 3, 1)     # [B, H, D, S]
    kT = np.asarray(key, np.float32).transpose(0, 2, 3, 1)       # [B, H, D, S]
    v = np.asarray(value, np.float32).transpose(0, 2, 1, 3)      # [B, H, S, D]
    mff = mf.astype(np.float32)
    kTz = kT * mff[:, None, None, :]
    vz = (v * mff[:, None, :, None]).astype(ml_dtypes.bfloat16)
    in_maps = []
    for c in range(N_CORES):
        qs = np.empty((8, P, S), np.float32)
        ks = np.empty((8, P, S), np.float32)
        vs = np.empty((8, S, P), ml_dtypes.bfloat16)
        for s in range(8):
            b, h = s // 2, 2 * c + (s % 2)
            w = qw[b]
            qs[s, :, :w] = qT[b, h, :, :w]
            ks[s, :, :w] = kTz[b, h, :, :w]
            vs[s, :w, :] = vz[b, h, :w, :]
        in_maps.append({"q": qs, "k": ks, "v": vs})
    return in_maps, mf


def assemble_output(results, mf):
    _, nkc, qw, _ = _plan(mf.astype(np.int32))
    mcount = np.array([nkc[b] * P - mf[b, :nkc[b] * P].sum() for b in range(B)],
                      np.float32)
    out = np.zeros((B, S, H * D), np.float32)
    for c in range(N_CORES):
        for s in range(8):
            b, h = s // 2, 2 * c + (s % 2)
            w = int(qw[b])
            oT = results[c]["out"][s][:, :w]                     # [D, w]
            sums = results[c]["sums_out"][s][:w] - mcount[b]     # [w]
            with np.errstate(divide="ignore", invalid="ignore"):
                scale = np.where(mf[b, :w], 1.0 / sums, 0.0)
            out[b, :w, h * D:(h + 1) * D] = (oT * scale[None, :]).T
    return out


def kernel(query, key, value, attn_mask):
    _, _, _, sched = _plan(attn_mask)
    nc = _get_program(sched)
    in_maps, mf = make_in_maps(query, key, value, attn_mask)
    res = run_bass_kernel_spmd(nc, in_maps, list(range(N_CORES)))
    return assemble_output(res.results, mf)


# revision 15
# speedup vs baseline: 1.2003x; 1.0878x over previous
"""Trainium2 Bass kernel: dense attention with key-padding mask (ColoAttention).

Math (per batch b, head h):
    scores = (Q @ K^T) / sqrt(D); masked keys -> -inf; softmax over keys;
    out = probs @ V; rows at masked query positions zeroed.

Implementation notes:
  - The mask is a contiguous valid prefix per batch (ragged sequences).  The
    host derives per-batch valid extents and compiles a program that only
    touches the valid key chunks / query columns (~50% of the dense work for
    the reference distribution).  Programs are cached per extent tuple.
  - Sharding balances the ragged work: every core gets 2 heads from EVERY
    batch (16 heads / 8 cores), so all cores run the identical schedule.
  - K and V rows at masked key positions are zeroed on the host, so scores
    at masked keys inside the last partial chunk are exactly 0, exp(0) = 1,
    and the per-row sum of exponentials just needs the (host-known)
    masked-key count subtracted.  Masked keys contribute 0 to probs @ V.
  - Scores are computed transposed (S^T[k, q] = K @ Q^T) so the exp output
    P^T[k, q] (bf16) is directly the moving operand for O'^T = V^T @ P^T.
  - Row sums: P^T chunks are pairwise tree-accumulated over k-chunks on the
    DVE (bf16), then one ones-vector matmul per q-sub-block reduces the 128
    partitions exactly in PSUM.  The PE stream is 2 passes of the score
    matrix (QK + PV) instead of 3.  The sums matmul + drain are deferred by
    one pipeline step so the PE never waits on the DVE fold; a dummy s-pool
    slot keeps the score-buffer rotation parity intact.
  - The device emits unnormalized O'^T and raw sums; the host applies
    qmask/(sums - mcount) and the final [D,S]->[S,D] transpose.
  - QK^T runs in float32r (full-rate fp32 on the PE), PV in bf16.
  - PSUM (8 banks): scores [128,1024] x2 = 4 banks (sums matmul rides this
    rotation), O' accum [128,1024] x2 = 4 banks.
"""

import numpy as np
import ml_dtypes
from contextlib import ExitStack

import concourse.bass as bass
import concourse.mybir as mybir
import concourse.tile as tile
from concourse import bacc
from concourse.bass_utils import run_bass_kernel_spmd

B, S, H, D = 4, 2048, 16, 128
N_CORES = 8
P = 128
SCALE = 1.0 / float(np.sqrt(np.float64(D)).astype(np.float32))


def _subs_of(qw: int):
    """Split the valid query width into <=1024-wide sub-blocks, multiples of
    256 so QK windows are 512-sized or a >=256 remainder (full-rate f32r)."""
    if qw <= 1024:
        return [(0, qw)]
    wa = min(1024, ((qw // 2 + 255) // 256) * 256)
    return [(0, wa), (wa, qw - wa)]


def _mm_windows(w: int):
    """512-wide (PSUM-bank sized) matmul windows covering [0, w)."""
    return [(ws, min(512, w - ws)) for ws in range(0, w, 512)]


def build_program(sched) -> bacc.Bacc:
    """sched: tuple of (nkc, qw) per slot, identical on every core."""
    f32 = mybir.dt.float32
    f32r = mybir.dt.float32r
    bf16 = mybir.dt.bfloat16
    Exp = mybir.ActivationFunctionType.Exp

    nc = bacc.Bacc("TRN2", target_bir_lowering=False, debug=False)
    q_d = nc.dram_tensor("q", [8, P, S], f32r, kind="ExternalInput").ap()
    k_d = nc.dram_tensor("k", [8, P, S], f32r, kind="ExternalInput").ap()
    v_d = nc.dram_tensor("v", [8, S, P], bf16, kind="ExternalInput").ap()
    out_d = nc.dram_tensor("out", [8, P, S], f32, kind="ExternalOutput").ap()
    sums_d = nc.dram_tensor("sums_out", [8, S], f32, kind="ExternalOutput").ap()

    # steps: (slot, sub_off, sub_w, kc)
    steps = []
    for s, (nkc, qw) in enumerate(sched):
        for (off, w) in _subs_of(qw):
            for kc in range(nkc):
                steps.append((s, off, w, kc))

    with tile.TileContext(nc) as tc:
        with ExitStack() as ctx:
            consts = ctx.enter_context(tc.tile_pool(name="consts", bufs=1))
            qkp = ctx.enter_context(tc.tile_pool(name="qkp", bufs=2))
            ptp = ctx.enter_context(tc.tile_pool(name="ptp", bufs=4))
            treep = ctx.enter_context(tc.tile_pool(name="treep", bufs=8))
            otp = ctx.enter_context(tc.tile_pool(name="otp", bufs=4))
            smp = ctx.enter_context(tc.tile_pool(name="smp", bufs=2))
            sps = ctx.enter_context(tc.tile_pool(name="sps", bufs=3, space="PSUM"))
            ops = ctx.enter_context(tc.tile_pool(name="ops", bufs=1, space="PSUM"))

            ones_b = consts.tile([P, 1], bf16, tag="ones")
            nc.gpsimd.memset(ones_b[:], 1.0)

            staged = {}

            def stage(s):
                if s in staged or s >= len(sched):
                    return
                nkc, qw = sched[s]
                q_sb = qkp.tile([P, qw], f32r, tag="q", name=f"q_{s}")
                k_sb = qkp.tile([P, nkc * P], f32r, tag="k", name=f"k_{s}")
                v_sb = qkp.tile([P, nkc, P], bf16, tag="v", name=f"v_{s}")
                # first compute needs k/v chunk 0 and the first q window
                nc.sync.dma_start(k_sb[:, 0:P], k_d[s, :, 0:P])
                nc.sync.dma_start(q_sb[:, 0:512], q_d[s, :, 0:512])
                nc.sync.dma_start(v_sb[:, 0, :], v_d[s, 0:P, :])
                for ws in range(512, qw, 512):
                    we = min(qw, ws + 512)
                    nc.sync.dma_start(q_sb[:, ws:we], q_d[s, :, ws:we])
                for kc in range(1, nkc):
                    nc.sync.dma_start(k_sb[:, kc * P:(kc + 1) * P],
                                      k_d[s, :, kc * P:(kc + 1) * P])
                    nc.sync.dma_start(v_sb[:, kc, :],
                                      v_d[s, kc * P:(kc + 1) * P, :])
                staged[s] = (q_sb, k_sb, v_sb)

            stage(0)
            stage(1)

            def emit_qk(i):
                s, off, w, kc = steps[i]
                stage(s + 1)
                q_sb, k_sb, _ = staged[s]
                s_ps = sps.tile([P, 1024], f32, tag="s", name=f"s_{i}")
                for (ws, ww) in _mm_windows(w):
                    nc.tensor.matmul(
                        s_ps[:, ws:ws + ww],
                        lhsT=k_sb[:, kc * P:(kc + 1) * P],
                        rhs=q_sb[:, off + ws:off + ws + ww],
                        start=True, stop=True)
                return s_ps

            # binary-counter tree accumulation of exp chunks (per sub-block)
            levels = [None] * 6

            def tree_push(pt, w, i):
                cur, lvl = pt, 0
                while levels[lvl] is not None:
                    prev = levels[lvl]
                    levels[lvl] = None
                    nt = treep.tile([P, 1024], bf16, tag="tree",
                                    name=f"tr_{i}_{lvl}")
                    nc.vector.tensor_add(nt[:, :w], prev[:, :w], cur[:, :w])
                    cur = nt
                    lvl += 1
                levels[lvl] = cur

            def tree_fold_partial(w, i):
                acc = None
                for lvl in range(6):
                    if levels[lvl] is None:
                        continue
                    if acc is None:
                        acc = levels[lvl]
                    else:
                        nt = treep.tile([P, 1024], bf16, tag="tree",
                                        name=f"tf_{i}_{lvl}")
                        nc.vector.tensor_add(nt[:, :w], acc[:, :w],
                                             levels[lvl][:, :w])
                        acc = nt
                    levels[lvl] = None
                return acc

            def make_tail(s, off, w, acc, i):
                """Deferred sums matmul + drains for a finished sub-block.
                Emitted one pipeline step later so the PE never waits on the
                DVE fold chain."""
                def tail():
                    sm_ps = sps.tile([P, 1024], f32, tag="s", name=f"sm_{i}")
                    # dummy allocations keep the s rotation parity (mod 3) so
                    # upcoming QKs land on buffers freed two-plus exps ago
                    sps.tile([P, 1024], f32, tag="s", name=f"sdua_{i}")
                    sps.tile([P, 1024], f32, tag="s", name=f"sdub_{i}")
                    for (ws, ww) in _mm_windows(w):
                        nc.tensor.matmul(sm_ps[0:1, ws:ws + ww],
                                         lhsT=ones_b[:],
                                         rhs=acc[:, ws:ws + ww],
                                         start=True, stop=True)
                    sm_st = smp.tile([1, 1024], f32, tag="smst",
                                     name=f"smst_{i}")
                    nc.vector.tensor_copy(out=sm_st[:, :w],
                                          in_=sm_ps[0:1, :w])
                    nc.sync.dma_start(sums_d[s, off:off + w], sm_st[:, :w])
                return tail

            pend = {0: emit_qk(0)}
            if len(steps) > 1:
                pend[1] = emit_qk(1)
            pend_tail = None
            ot_ps = None
            prefold = None
            for i, (s, off, w, kc) in enumerate(steps):
                nkc, qw = sched[s]
                q_sb, k_sb, v_sb = staged[s]
                if kc == 0:
                    ot_ps = ops.tile([P, 1024], f32, tag="o", name=f"ot_{i}")
                    prefold = None
                pt = ptp.tile([P, 1024], bf16, tag="pt", name=f"pt_{i}")
                s_ps = pend.pop(i)
                nc.scalar.activation(pt[:, :w], s_ps[:, :w], Exp, scale=SCALE)
                if i + 2 < len(steps):
                    pend[i + 2] = emit_qk(i + 2)
                last = kc == nkc - 1
                for (ws, ww) in _mm_windows(w):
                    nc.tensor.matmul(
                        ot_ps[:, ws:ws + ww],
                        lhsT=v_sb[:, kc, :],
                        rhs=pt[:, ws:ws + ww],
                        start=(kc == 0), stop=last)
                if pend_tail is not None:
                    pend_tail()
                    pend_tail = None
                if not last:
                    tree_push(pt, w, i)
                    if kc == nkc - 2:
                        prefold = tree_fold_partial(w, i)
                    continue

                # ---- sub-block tail ----
                if nkc > 1:
                    acc = treep.tile([P, 1024], bf16, tag="tree",
                                     name=f"acc_{i}")
                    nc.vector.tensor_add(acc[:, :w], prefold[:, :w],
                                         pt[:, :w])
                else:
                    acc = pt
                # drain O'^T, split per bank-window so the single O' PSUM
                # buffer frees earlier (GpSimd cannot touch PSUM)
                o_st = otp.tile([P, 1024], f32, tag="ost", name=f"ost_{i}")
                for (ws, ww) in _mm_windows(w):
                    nc.vector.tensor_copy(out=o_st[:, ws:ws + ww],
                                          in_=ot_ps[:, ws:ws + ww])
                    nc.sync.dma_start(out_d[s, :, off + ws:off + ws + ww],
                                      o_st[:, ws:ws + ww])
                pend_tail = make_tail(s, off, w, acc, i)
            if pend_tail is not None:
                pend_tail()

    nc.compile()
    return nc


_PROG_CACHE: dict = {}


def _get_program(sched) -> bacc.Bacc:
    if sched not in _PROG_CACHE:
        _PROG_CACHE[sched] = build_program(sched)
    return _PROG_CACHE[sched]


def _plan(attn_mask):
    mf = (np.asarray(attn_mask) > 0)
    any_valid = mf.any(axis=1)
    last_plus1 = np.where(any_valid, S - np.argmax(mf[:, ::-1], axis=1), 1)
    nkc = np.maximum(1, (last_plus1 + P - 1) // P).astype(int)   # [B]
    qw = nkc * P
    # slot s -> batch s//2 (two head-slots per batch per core)
    sched = tuple((int(nkc[s // 2]), int(qw[s // 2])) for s in range(8))
    return mf, nkc, qw, sched


def make_in_maps(query, key, value, attn_mask):
    mf, nkc, qw, sched = _plan(attn_mask)
    # device wants q/k as [slot, D, S] (pre-transposed), v as [slot, S, D]
    qT = np.asarray(query, np.float32).transpose(0, 2, 3, 1)     # [B, H, D, S]
    kT = np.asarray(key, np.float32).transpose(0, 2, 3, 1)       # [B, H, D, S]
    v = np.asarray(value, np.float32).transpose(0, 2, 1, 3)      # [B, H, S, D]
    mff = mf.astype(np.float32)
    kTz = kT * mff[:, None, None, :]
    vz = (v * mff[:, None, :, None]).astype(ml_dtypes.bfloat16)
    in_maps = []
    for c in range(N_CORES):
        qs = np.empty((8, P, S), np.float32)
        ks = np.empty((8, P, S), np.float32)
        vs = np.empty((8, S, P), ml_dtypes.bfloat16)
        for s in range(8):
            b, h = s // 2, 2 * c + (s % 2)
            w = qw[b]
            qs[s, :, :w] = qT[b, h, :, :w]
            ks[s, :, :w] = kTz[b, h, :, :w]
            vs[s, :w, :] = vz[b, h, :w, :]
        in_maps.append({"q": qs, "k": ks, "v": vs})
    return in_maps, mf


def assemble_output(results, mf):
    _, nkc, qw, _ = _plan(mf.astype(np.int32))
    mcount = np.array([nkc[b] * P - mf[b, :nkc[b] * P].sum() for b in range(B)],
                      np.float32)
    out = np.zeros((B, S, H * D), np.float32)
    for c in range(N_CORES):
        for s in range(8):
            b, h = s // 2, 2 * c + (s % 2)
            w = int(qw[b])
            oT = results[c]["out"][s][:, :w]                     # [D, w]
            sums = results[c]["sums_out"][s][:w] - mcount[b]     # [w]
            with np.errstate(divide="ignore", invalid="ignore"):
                scale = np.where(mf[b, :w], 1.0 / sums, 0.0)
            out[b, :w, h * D:(h + 1) * D] = (oT * scale[None, :]).T
    return out


def kernel(query, key, value, attn_mask):
    _, _, _, sched = _plan(attn_mask)
    nc = _get_program(sched)
    in_maps, mf = make_in_maps(query, key, value, attn_mask)
    res = run_bass_kernel_spmd(nc, in_maps, list(range(N_CORES)))
    return assemble_output(res.results, mf)


# revision 19
# speedup vs baseline: 1.3530x; 1.1272x over previous
"""Trainium2 Bass kernel: dense attention with key-padding mask (ColoAttention).

Math (per batch b, head h):
    scores = (Q @ K^T) / sqrt(D); masked keys -> -inf; softmax over keys;
    out = probs @ V; rows at masked query positions zeroed.

Implementation notes:
  - The mask is a contiguous valid prefix per batch (ragged sequences).  The
    host derives per-batch valid extents and compiles a program that only
    touches the valid key chunks / query columns (~50% of the dense work for
    the reference distribution).  Programs are cached per extent tuple.
  - Sharding balances the ragged work: every core gets 2 heads from EVERY
    batch (16 heads / 8 cores), so all cores run the identical schedule.
  - K and V rows at masked key positions are zeroed on the host, so scores
    at masked keys inside the last partial chunk are exactly 0, exp(0) = 1,
    and the per-row sum of exponentials just needs the (host-known)
    masked-key count subtracted.  Masked keys contribute 0 to probs @ V.
  - Scores are computed transposed (S^T[k, q] = K @ Q^T) so the exp output
    P^T[k, q] (bf16) is directly the moving operand for O'^T = V^T @ P^T.
  - Row sums: P^T chunks are pairwise tree-accumulated over k-chunks on the
    DVE (bf16), then one ones-vector matmul per q-sub-block reduces the 128
    partitions exactly in PSUM.  The PE stream is 2 passes of the score
    matrix (QK + PV) instead of 3.  The sums matmul + drain are deferred by
    one pipeline step so the PE never waits on the DVE fold; a dummy s-pool
    slot keeps the score-buffer rotation parity intact.
  - The device emits unnormalized O'^T and raw sums; the host applies
    qmask/(sums - mcount) and the final [D,S]->[S,D] transpose.
  - QK^T runs in float32r (full-rate fp32 on the PE), PV in bf16.
  - PSUM (8 banks): scores [128,1024] x2 = 4 banks (sums matmul rides this
    rotation), O' accum [128,1024] x2 = 4 banks.
"""

import numpy as np
import ml_dtypes
from contextlib import ExitStack

import concourse.bass as bass
import concourse.mybir as mybir
import concourse.tile as tile
from concourse import bacc
from concourse.bass_utils import run_bass_kernel_spmd

B, S, H, D = 4, 2048, 16, 128
N_CORES = 8
P = 128
SCALE = 1.0 / float(np.sqrt(np.float64(D)).astype(np.float32))


def _subs_of(qw: int):
    """Split the valid query width into <=1024-wide sub-blocks, multiples of
    256 so QK windows are 512-sized or a >=256 remainder (full-rate f32r)."""
    if qw <= 1024:
        return [(0, qw)]
    wa = min(1024, ((qw // 2 + 255) // 256) * 256)
    return [(0, wa), (wa, qw - wa)]


def _mm_windows(w: int):
    """512-wide (PSUM-bank sized) matmul windows covering [0, w)."""
    return [(ws, min(512, w - ws)) for ws in range(0, w, 512)]


def build_program(sched) -> bacc.Bacc:
    """sched: tuple of (nkc, qw) per slot, identical on every core."""
    f32 = mybir.dt.float32
    f32r = mybir.dt.float32r
    bf16 = mybir.dt.bfloat16
    Exp = mybir.ActivationFunctionType.Exp

    nc = bacc.Bacc("TRN2", target_bir_lowering=False, debug=False)
    q_d = nc.dram_tensor("q", [8, P, S], f32r, kind="ExternalInput").ap()
    k_d = nc.dram_tensor("k", [8, P, S], f32r, kind="ExternalInput").ap()
    v_d = nc.dram_tensor("v", [8, S, P], bf16, kind="ExternalInput").ap()
    out_d = nc.dram_tensor("out", [8, P, S], f32, kind="ExternalOutput").ap()
    sums_d = nc.dram_tensor("sums_out", [8, S], f32, kind="ExternalOutput").ap()

    # steps: (slot, sub_off, sub_w, kc)
    steps = []
    for s, (nkc, qw) in enumerate(sched):
        for (off, w) in _subs_of(qw):
            for kc in range(nkc):
                steps.append((s, off, w, kc))

    with tile.TileContext(nc) as tc:
        with ExitStack() as ctx:
            consts = ctx.enter_context(tc.tile_pool(name="consts", bufs=1))
            qkp = ctx.enter_context(tc.tile_pool(name="qkp", bufs=2))
            ptp = ctx.enter_context(tc.tile_pool(name="ptp", bufs=4))
            treep = ctx.enter_context(tc.tile_pool(name="treep", bufs=8))
            otp = ctx.enter_context(tc.tile_pool(name="otp", bufs=4))
            smp = ctx.enter_context(tc.tile_pool(name="smp", bufs=2))
            sps = ctx.enter_context(tc.tile_pool(name="sps", bufs=3, space="PSUM"))
            ops = ctx.enter_context(tc.tile_pool(name="ops", bufs=1, space="PSUM"))

            ones_b = consts.tile([P, 1], bf16, tag="ones")
            nc.gpsimd.memset(ones_b[:], 1.0)

            staged = {}

            def stage(s):
                if s in staged or s >= len(sched):
                    return
                nkc, qw = sched[s]
                q_sb = qkp.tile([P, qw], f32r, tag="q", name=f"q_{s}")
                k_sb = qkp.tile([P, nkc * P], f32r, tag="k", name=f"k_{s}")
                v_sb = qkp.tile([P, nkc, P], bf16, tag="v", name=f"v_{s}")
                # first compute needs k/v chunk 0 and the first q window;
                # the rest arrives as one large transfer each (the sync
                # sequencer's per-DMA issue cost is ~0.6us, so keep few DMAs)
                nc.sync.dma_start(k_sb[:, 0:P], k_d[s, :, 0:P])
                nc.sync.dma_start(q_sb[:, 0:512], q_d[s, :, 0:512])
                nc.sync.dma_start(v_sb[:, 0, :], v_d[s, 0:P, :])
                if qw > 512:
                    nc.sync.dma_start(q_sb[:, 512:qw], q_d[s, :, 512:qw])
                if nkc > 1:
                    nc.sync.dma_start(k_sb[:, P:nkc * P], k_d[s, :, P:nkc * P])
                    nc.sync.dma_start(
                        v_sb[:, 1:nkc, :],
                        v_d[s, P:nkc * P, :].rearrange("(t r) d -> r t d", r=P))
                staged[s] = (q_sb, k_sb, v_sb)

            stage(0)
            stage(1)

            def emit_qk(i):
                s, off, w, kc = steps[i]
                stage(s + 1)
                q_sb, k_sb, _ = staged[s]
                s_ps = sps.tile([P, 1024], f32, tag="s", name=f"s_{i}")
                for (ws, ww) in _mm_windows(w):
                    nc.tensor.matmul(
                        s_ps[:, ws:ws + ww],
                        lhsT=k_sb[:, kc * P:(kc + 1) * P],
                        rhs=q_sb[:, off + ws:off + ws + ww],
                        start=True, stop=True)
                return s_ps

            # binary-counter tree accumulation of exp chunks (per sub-block)
            levels = [None] * 6

            def tree_push(pt, w, i):
                cur, lvl = pt, 0
                while levels[lvl] is not None:
                    prev = levels[lvl]
                    levels[lvl] = None
                    nt = treep.tile([P, 1024], bf16, tag="tree",
                                    name=f"tr_{i}_{lvl}")
                    nc.vector.tensor_add(nt[:, :w], prev[:, :w], cur[:, :w])
                    cur = nt
                    lvl += 1
                levels[lvl] = cur

            def tree_fold_partial(w, i):
                acc = None
                for lvl in range(6):
                    if levels[lvl] is None:
                        continue
                    if acc is None:
                        acc = levels[lvl]
                    else:
                        nt = treep.tile([P, 1024], bf16, tag="tree",
                                        name=f"tf_{i}_{lvl}")
                        nc.vector.tensor_add(nt[:, :w], acc[:, :w],
                                             levels[lvl][:, :w])
                        acc = nt
                    levels[lvl] = None
                return acc

            def make_tail(s, off, w, acc, i):
                """Deferred sums matmul + drains for a finished sub-block.
                Emitted one pipeline step later so the PE never waits on the
                DVE fold chain."""
                def tail():
                    sm_ps = sps.tile([P, 1024], f32, tag="s", name=f"sm_{i}")
                    # dummy allocations keep the s rotation parity (mod 3) so
                    # upcoming QKs land on buffers freed two-plus exps ago
                    sps.tile([P, 1024], f32, tag="s", name=f"sdua_{i}")
                    sps.tile([P, 1024], f32, tag="s", name=f"sdub_{i}")
                    for (ws, ww) in _mm_windows(w):
                        nc.tensor.matmul(sm_ps[0:1, ws:ws + ww],
                                         lhsT=ones_b[:],
                                         rhs=acc[:, ws:ws + ww],
                                         start=True, stop=True)
                    sm_st = smp.tile([1, 1024], f32, tag="smst",
                                     name=f"smst_{i}")
                    nc.vector.tensor_copy(out=sm_st[:, :w],
                                          in_=sm_ps[0:1, :w])
                    nc.gpsimd.dma_start(sums_d[s, off:off + w], sm_st[:, :w])
                return tail

            pend = {0: emit_qk(0)}
            if len(steps) > 1:
                pend[1] = emit_qk(1)
            pend_tail = []
            ot_ps = None
            prefold = None
            for i, (s, off, w, kc) in enumerate(steps):
                nkc, qw = sched[s]
                q_sb, k_sb, v_sb = staged[s]
                if kc == 0:
                    ot_ps = ops.tile([P, 1024], f32, tag="o", name=f"ot_{i}")
                    prefold = None
                pt = ptp.tile([P, 1024], bf16, tag="pt", name=f"pt_{i}")
                s_ps = pend.pop(i)
                nc.scalar.activation(pt[:, :w], s_ps[:, :w], Exp, scale=SCALE)
                if i + 2 < len(steps):
                    pend[i + 2] = emit_qk(i + 2)
                last = kc == nkc - 1
                for (ws, ww) in _mm_windows(w):
                    nc.tensor.matmul(
                        ot_ps[:, ws:ws + ww],
                        lhsT=v_sb[:, kc, :],
                        rhs=pt[:, ws:ws + ww],
                        start=(kc == 0), stop=last)
                while pend_tail and pend_tail[0][0] <= i:
                    pend_tail.pop(0)[1]()
                if not last:
                    tree_push(pt, w, i)
                    if kc == nkc - 2:
                        prefold = tree_fold_partial(w, i)
                    continue

                # ---- sub-block tail ----
                # drain O'^T first, split per bank-window, so the single O'
                # PSUM buffer frees as early as possible (GpSimd cannot
                # touch PSUM, so the copies run on the DVE)
                o_st = otp.tile([P, 1024], f32, tag="ost", name=f"ost_{i}")
                for (ws, ww) in _mm_windows(w):
                    nc.vector.tensor_copy(out=o_st[:, ws:ws + ww],
                                          in_=ot_ps[:, ws:ws + ww])
                    nc.gpsimd.dma_start(out_d[s, :, off + ws:off + ws + ww],
                                        o_st[:, ws:ws + ww])
                if nkc > 1:
                    acc = treep.tile([P, 1024], bf16, tag="tree",
                                     name=f"acc_{i}")
                    nc.vector.tensor_add(acc[:, :w], prefold[:, :w],
                                         pt[:, :w])
                else:
                    acc = pt
                pend_tail.append((i + 2, make_tail(s, off, w, acc, i)))
            while pend_tail:
                pend_tail.pop(0)[1]()

    nc.compile()
    return nc


_PROG_CACHE: dict = {}


def _get_program(sched) -> bacc.Bacc:
    if sched not in _PROG_CACHE:
        _PROG_CACHE[sched] = build_program(sched)
    return _PROG_CACHE[sched]


def _plan(attn_mask):
    mf = (np.asarray(attn_mask) > 0)
    any_valid = mf.any(axis=1)
    last_plus1 = np.where(any_valid, S - np.argmax(mf[:, ::-1], axis=1), 1)
    nkc = np.maximum(1, (last_plus1 + P - 1) // P).astype(int)   # [B]
    qw = nkc * P
    # slot s -> batch s//2 (two head-slots per batch per core)
    sched = tuple((int(nkc[s // 2]), int(qw[s // 2])) for s in range(8))
    return mf, nkc, qw, sched


def make_in_maps(query, key, value, attn_mask):
    mf, nkc, qw, sched = _plan(attn_mask)
    # device wants q/k as [slot, D, S] (pre-transposed), v as [slot, S, D]
    qT = np.asarray(query, np.float32).transpose(0, 2, 3, 1)     # [B, H, D, S]
    kT = np.asarray(key, np.float32).transpose(0, 2, 3, 1)       # [B, H, D, S]
    v = np.asarray(value, np.float32).transpose(0, 2, 1, 3)      # [B, H, S, D]
    mff = mf.astype(np.float32)
    kTz = kT * mff[:, None, None, :]
    vz = (v * mff[:, None, :, None]).astype(ml_dtypes.bfloat16)
    in_maps = []
    for c in range(N_CORES):
        qs = np.empty((8, P, S), np.float32)
        ks = np.empty((8, P, S), np.float32)
        vs = np.empty((8, S, P), ml_dtypes.bfloat16)
        for s in range(8):
            b, h = s // 2, 2 * c + (s % 2)
            w = qw[b]
            qs[s, :, :w] = qT[b, h, :, :w]
            ks[s, :, :w] = kTz[b, h, :, :w]
            vs[s, :w, :] = vz[b, h, :w, :]
        in_maps.append({"q": qs, "k": ks, "v": vs})
    return in_maps, mf


def assemble_output(results, mf):
    _, nkc, qw, _ = _plan(mf.astype(np.int32))
    mcount = np.array([nkc[b] * P - mf[b, :nkc[b] * P].sum() for b in range(B)],
                      np.float32)
    out = np.zeros((B, S, H * D), np.float32)
    for c in range(N_CORES):
        for s in range(8):
            b, h = s // 2, 2 * c + (s % 2)
            w = int(qw[b])
            oT = results[c]["out"][s][:, :w]                     # [D, w]
            sums = results[c]["sums_out"][s][:w] - mcount[b]     # [w]
            with np.errstate(divide="ignore", invalid="ignore"):
                scale = np.where(mf[b, :w], 1.0 / sums, 0.0)
            out[b, :w, h * D:(h + 1) * D] = (oT * scale[None, :]).T
    return out


def kernel(query, key, value, attn_mask):
    _, _, _, sched = _plan(attn_mask)
    nc = _get_program(sched)
    in_maps, mf = make_in_maps(query, key, value, attn_mask)
    res = run_bass_kernel_spmd(nc, in_maps, list(range(N_CORES)))
    return assemble_output(res.results, mf)
